# revision 15
# baseline (speedup 1.0000x reference)
"""BERT self-attention (BS=4, SEQ=2048, HID=768, NH=12) on 8 NeuronCores.

Sharding: core c -> batch b = c//2, head-group g = c%2 (6 heads each).

Per core, for its batch element and 6 heads (3 pairs j=0..2):
  Q^T/K^T[j] = W_j @ X^T + b  in [d, q] layout (d on partitions), bias
               folded on the DVE during the PSUM->SBUF copy.
  V          = X @ Wv^T + bv  in [k, d] layout, rows scaled by mask m[k],
               projected inline during the first attention loop.
  S^T        = K^T.T-free matmul -> [k_block, q] scores in PSUM
               (two heads row-tiled at PE rows 0-63 / 64-127).
  P^T        = exp(S^T / 8)   (ACT, PSUM->SBUF f16; mask folded into V)
  ctx^T      = V'.T @ P^T accumulated over k blocks (heads col-tiled into
               rows 0-63 / 64-127 of one PSUM tile).
  denom      = mask-column m=1 matmuls, 4-way col-tiled, into a 1-bank
               PSUM tile.
  out        = ctx^T * broadcast(1/denom), f16, DMA'd per q-half.

The emission is software-pipelined: scores/exp for k-block t are emitted
before ctx/denom for t-1, so the ACT engine (the exp throughput bound)
never waits on the PE and vice versa.  QK projections for head-pair j+1
are interleaved into attention loop j.

Host does input transposes (free), sharding, and the final [d,q]->[q,d]
untranspose + concat.
"""

import numpy as np

import concourse.bass as bass
import concourse.tile as tile
from concourse import bacc
from concourse import mybir
from concourse.bass_utils import run_bass_kernel_spmd

F32 = mybir.dt.float32
F16 = mybir.dt.float16
DT_MM = F16          # dtype for matmul operands
DT_NP = np.float16   # matching numpy dtype for host-side input prep

BS, SEQ, HID, NH, HD = 4, 2048, 768, 12, 64
NCORES = 8
HPC = 6          # heads per core
FCH = 6          # 128-row chunks of the 768 contraction dim
DSH = HPC * HD   # 384 output features per core


def _body(tc, xt_d, wq_d, wk_d, wv_d, bqc_d, mt_d, ot_d):
    nc = tc.nc
    Exp = mybir.ActivationFunctionType.Exp

    with tc.tile_pool(name="persist", bufs=1) as P, \
         tc.tile_pool(name="work", bufs=1) as W, \
         tc.tile_pool(name="ps", bufs=1, space="PSUM") as PS:
        # Warm the exp table set ASAP (overlaps the input DMAs).
        dummy = P.tile([1, 1], F32, tag="dummy")
        nc.vector.memset(dummy, 0.0)
        nc.scalar.activation(out=dummy, in_=dummy, func=Exp)

        # ---------------- input DMAs ----------------
        xts = []
        for f in range(FCH):
            t = P.tile([128, SEQ], DT_MM, tag=f"x{f}", name=f"x{f}")
            nc.sync.dma_start(out=t, in_=xt_d[f * 128:(f + 1) * 128, :])
            xts.append(t)

        # Q/K weights arrive as per-head-pair [128,128] slices so the first
        # DMA wave (everything the lead-in projections need) is minimal;
        # j=1,2 slices are emitted last and may arrive during attention j=0.
        wmap = {"q": [[None] * 3 for _ in range(FCH)],
                "k": [[None] * 3 for _ in range(FCH)]}
        for dram, nm in ((wq_d, "q"), (wk_d, "k")):
            for f in range(FCH):
                t = P.tile([128, 128], DT_MM, tag=f"w{nm}{f}0", name=f"w{nm}{f}0")
                nc.sync.dma_start(out=t, in_=dram[f * 128:(f + 1) * 128, 0:128])
                wmap[nm][f][0] = t

        bqc = P.tile([128, 6], F32, tag="bqc")
        nc.sync.dma_start(out=bqc, in_=bqc_d[:, :])

        mtile = P.tile([128, 16], DT_MM, tag="mtile")
        nc.sync.dma_start(out=mtile, in_=mt_d[:, :])
        mtf = P.tile([128, 16], F32, tag="mtf")
        nc.vector.tensor_copy(out=mtf, in_=mtile)

        xt1 = P.tile([1, SEQ], DT_MM, tag="x6")
        nc.sync.dma_start(out=xt1, in_=xt_d[768:769, :])

        wv = []
        for f in range(FCH):
            t = P.tile([128, DSH], DT_MM, tag=f"wv{f}", name=f"wv{f}")
            nc.sync.dma_start(out=t, in_=wv_d[f * 128:(f + 1) * 128, :])
            wv.append(t)
        wvb = P.tile([1, DSH], DT_MM, tag="wvb")
        nc.sync.dma_start(out=wvb, in_=wv_d[768:769, :])

        for dram, nm in ((wq_d, "q"), (wk_d, "k")):
            for j in (1, 2):
                for f in range(FCH):
                    t = P.tile([128, 128], DT_MM, tag=f"w{nm}{f}{j}",
                               name=f"w{nm}{f}{j}")
                    nc.sync.dma_start(
                        out=t, in_=dram[f * 128:(f + 1) * 128,
                                        j * 128:(j + 1) * 128])
                    wmap[nm][f][j] = t

        # ---------------- persistent compute tiles ----------------
        qt = [P.tile([128, SEQ], DT_MM, tag=f"qt{j}", name=f"qt{j}") for j in range(3)]
        kt = [P.tile([128, SEQ], DT_MM, tag=f"kt{j}", name=f"kt{j}") for j in range(3)]
        vt = P.tile([128, 16, DSH], DT_MM, tag="vt")

        def proj_part(nm, j, qc, tag, frange, ps=None, bufs=1):
            """Part of a [128,512] chunk of Q^T[j] or K^T[j]: accumulating
            matmuls over frange; on the last one, a DVE copy folds the
            per-partition bias.  Returns the PSUM tile for continuation."""
            qs = slice(qc * 512, (qc + 1) * 512)
            if ps is None:
                ps = PS.tile([128, 512], F32, tag=tag, name=f"pj{nm}{j}{qc}",
                             bufs=bufs)
            for f in frange:
                nc.tensor.matmul(ps, lhsT=wmap[nm][f][j], rhs=xts[f][:, qs],
                                 start=(f == 0), stop=(f == FCH - 1))
            if frange[-1] == FCH - 1:
                dst = qt[j] if nm == "q" else kt[j]
                bcol = j if nm == "q" else 3 + j
                nc.vector.tensor_scalar_add(out=dst[:, qs], in0=ps,
                                            scalar1=bqc[:, bcol:bcol + 1])
            return ps

        def proj_chunk(nm, j, qc, tag, bufs=1):
            proj_part(nm, j, qc, tag, range(FCH), bufs=bufs)

        def v_chunk(kb):
            """V rows for k-block kb, all 6 heads, mask-scaled into vt."""
            ks = slice(kb * 128, (kb + 1) * 128)
            vps = PS.tile([128, DSH], F32, tag="x", name=f"vps{kb}")
            for f in range(FCH):
                nc.tensor.matmul(vps, lhsT=xts[f][:, ks], rhs=wv[f],
                                 start=(f == 0), stop=False)
            nc.tensor.matmul(vps, lhsT=xt1[:, ks], rhs=wvb,
                             start=False, stop=True)
            nc.vector.tensor_scalar_mul(out=vt[:, kb, :], in0=vps,
                                        scalar1=mtf[:, kb:kb + 1])

        # PE warmup: dense dummy matmuls during the input DMA wait so the
        # HAM clock gate opens (1.2 -> 2.4 GHz) before real work starts.
        warm = P.tile([128, 512], DT_MM, tag="warm")
        nc.vector.memset(warm, 0.0)

        def filler(n, tags=("x", "s")):
            """Always-ready dummy matmuls: bridge PE idle windows so the
            HAM clock gate never re-throttles to 1.2 GHz."""
            for t in range(n):
                wps = PS.tile([128, 512], F32, tag=tags[t % len(tags)],
                              name="wps", bufs=2 if tags[t % len(tags)] == "s" else 1)
                nc.tensor.matmul(wps, lhsT=warm[:, 0:128], rhs=warm,
                                 start=True, stop=True)

        filler(20)

        # Lead-in: project Q^T[0], K^T[0] using the (idle) scores buffers
        # so the chunks double-buffer.
        for nm in ("q", "k"):
            for qc in range(4):
                proj_chunk(nm, 0, qc, "s", bufs=2)

        # ---------------- attention ----------------
        # PSUM tags: s 2x[128,1024] (4 banks) + c [128,1024] (2 banks) +
        # d [97,512] (1 bank) + x [128,512] (1 bank) = 8 banks.
        pending = []   # deferred bc/mul/dma work from the previous drain
        for j in range(3):
            heads = (2 * j, 2 * j + 1)
            # projection chunks for the next head pair, spread over slots
            chunks = []
            if j < 2:
                chunks = [(nm, j + 1, qc) for nm in ("q", "k") for qc in range(4)]
            for qh in range(2):
                q0 = qh * 1024

                cab = PS.tile([128, 1024], F32, tag="c", name="cab")
                dnt = PS.tile([97, 512], F32, tag="d", name="dnt")

                def ctx_dn(ppab, pkb):
                    st, sp_ = (pkb == 0), (pkb == 15)
                    for qq in range(2):
                        osl = slice(qq * 512, (qq + 1) * 512)
                        for i in range(2):
                            nc.tensor.matmul(
                                cab[64 * i:64 * (i + 1), osl],
                                lhsT=vt[:, pkb, heads[i] * 64:(heads[i] + 1) * 64],
                                rhs=ppab[i][:, osl], start=st, stop=sp_,
                                skip_group_check=True)
                    for idx, (i, qq) in enumerate(((0, 0), (1, 0), (0, 1), (1, 1))):
                        osl = slice(qq * 512, (qq + 1) * 512)
                        r = 32 * idx
                        nc.tensor.matmul(dnt[r:r + 1, :],
                                         lhsT=mtile[:, pkb:pkb + 1],
                                         rhs=ppab[i][:, osl],
                                         start=st, stop=sp_,
                                         tile_position=(0, r),
                                         skip_group_check=True)

                LAG = 3  # ctx/dn trail scores/exp by 3 slots so they are
                # never not-ready when they reach the PE (the 4-deep wait
                # queue would otherwise block the engine at loop boundaries)
                lagq = []
                half_state = None
                for kb in range(16):
                    ks = slice(kb * 128, (kb + 1) * 128)
                    # scores for kb (row-tiled head pairs, ping-pong bufs)
                    sab = [PS.tile([128, 1024], F32, tag="s", name="sab", bufs=2)
                           for _ in range(2)]
                    for qq in range(2):
                        qs = slice(q0 + qq * 512, q0 + (qq + 1) * 512)
                        osl = slice(qq * 512, (qq + 1) * 512)
                        for i in range(2):
                            rows = slice(64 * i, 64 * (i + 1))
                            nc.tensor.matmul(sab[i][:, osl],
                                             lhsT=kt[j][rows, ks],
                                             rhs=qt[j][rows, qs],
                                             start=True, stop=True)
                    # exp for kb
                    pab = [W.tile([128, 1024], DT_MM, tag="p", name="ptile", bufs=8)
                           for _ in range(2)]
                    for i in range(2):
                        nc.scalar.activation(out=pab[i], in_=sab[i], func=Exp,
                                             scale=0.125)
                    # V projection for kb (consumed by ctx at slot kb+LAG)
                    if j == 0 and qh == 0:
                        v_chunk(kb)
                    # lagged ctx + denominators
                    lagq.append((pab, kb))
                    if len(lagq) > LAG:
                        ctx_dn(*lagq.pop(0))
                    # deferred normalize/store work from the previous drain
                    if pending and 2 <= kb <= 5:
                        pending.pop(0)()
                    # interleaved projection work for j+1: 3 matmuls per
                    # slot (half a chunk) keeps every slot under the ACT
                    # exp pace
                    if half_state is not None:
                        nm, pj, qc, hps = half_state
                        proj_part(nm, pj, qc, "x", range(3, FCH), ps=hps)
                        half_state = None
                    elif chunks and kb % 2 == 1 and not (j == 0 and qh == 0):
                        nm, pj, qc = chunks.pop(0)
                        hps = proj_part(nm, pj, qc, "x", range(0, 3))
                        half_state = (nm, pj, qc, hps)
                # flush the lagged ctx/dn (also bridges the loop boundary
                # so the PE never idles long enough to re-throttle)
                for ent in lagq:
                    ctx_dn(*ent)
                if half_state is not None:
                    nm, pj, qc, hps = half_state
                    proj_part(nm, pj, qc, "x", range(3, FCH), ps=hps)
                    half_state = None

                # drain (DVE only): free cab/dnt quickly, compute 1/denom.
                # The bc-broadcast matmuls + final multiplies + store DMAs
                # are deferred into slots 2-5 of the next loop so no
                # long-waiting matmul ever clogs the PE wait queue at the
                # loop boundary.
                cts, r16s = [], []
                for i in range(2):
                    ct = W.tile([64, 1024], F32, tag="ct", name="ct", bufs=4)
                    nc.vector.tensor_copy(out=ct, in_=cab[64 * i:64 * (i + 1), :])
                    cts.append(ct)
                    rdf = W.tile([1, 1024], F32, tag="rd", name="rdf", bufs=2)
                    nc.vector.tensor_copy(out=rdf[:, 0:512],
                                          in_=dnt[32 * i:32 * i + 1, :])
                    nc.vector.tensor_copy(out=rdf[:, 512:1024],
                                          in_=dnt[64 + 32 * i:64 + 32 * i + 1, :])
                    rcp = W.tile([1, 1024], F32, tag="rcp", name="rcp", bufs=2)
                    nc.vector.reciprocal_approx_fast(out=rcp, in_=rdf)
                    rcp16 = W.tile([1, 1024], DT_MM, tag="rcp16", name="rcp16", bufs=2)
                    nc.vector.tensor_copy(out=rcp16, in_=rcp)
                    r16s.append(rcp16)
                osts = [W.tile([64, 1024], DT_MM, tag="os", name="ost", bufs=4)
                        for _ in range(2)]

                def normalize_store(i, half, q0=q0, heads=heads,
                                    cts=cts, r16s=r16s, osts=osts):
                    osl = slice(half * 512, (half + 1) * 512)
                    bc = PS.tile([64, 512], F32, tag="x", name="bc")
                    nc.tensor.matmul(bc, lhsT=xt1[:, 0:64],
                                     rhs=r16s[i][:, osl],
                                     start=True, stop=True)
                    nc.vector.tensor_mul(out=osts[i][:, osl],
                                         in0=cts[i][:, osl], in1=bc)
                    if half == 1:
                        nc.sync.dma_start(out=ot_d[heads[i]][:, q0:q0 + 1024],
                                          in_=osts[i])

                pending = [lambda i=i, half=half: normalize_store(i, half)
                           for i in range(2) for half in range(2)]
        # tail: final drain work has no following loop
        for fn in pending:
            fn()


def build_nc():
    nc = bacc.Bacc("TRN2")
    xt_d = nc.declare_dram_parameter("xt", [HID + 1, SEQ], DT_MM, isOutput=False)
    wq_d = nc.declare_dram_parameter("wqT", [HID, DSH], DT_MM, isOutput=False)
    wk_d = nc.declare_dram_parameter("wkT", [HID, DSH], DT_MM, isOutput=False)
    wv_d = nc.declare_dram_parameter("wvT", [HID + 1, DSH], DT_MM, isOutput=False)
    bqc_d = nc.declare_dram_parameter("bqc", [128, 6], F32, isOutput=False)
    mt_d = nc.declare_dram_parameter("mt", [128, 16], DT_MM, isOutput=False)
    ot_d = nc.declare_dram_parameter("OT", [HPC, HD, SEQ], DT_MM, isOutput=True)
    with tile.TileContext(nc) as tc:
        _body(tc, xt_d, wq_d, wk_d, wv_d, bqc_d, mt_d, ot_d)
    nc.finalize()
    return nc


_NC_CACHE = None


def _get_nc():
    global _NC_CACHE
    if _NC_CACHE is None:
        _NC_CACHE = build_nc()
    return _NC_CACHE


def make_in_maps(hidden_states, attention_mask, Wq, bq, Wk, bk, Wv, bv):
    in_maps = []
    for c in range(NCORES):
        b, g = c // 2, c % 2
        hs = slice(g * DSH, (g + 1) * DSH)
        xt = np.empty((HID + 1, SEQ), DT_NP)
        xt[:HID] = hidden_states[b].T
        xt[HID] = 1.0
        m = (attention_mask[b, 0, 0] > -1).astype(DT_NP)
        mt = np.ascontiguousarray(m.reshape(16, 128).T)

        wva = np.empty((HID + 1, DSH), DT_NP)
        wva[:HID] = Wv[hs, :].T
        wva[HID] = bv[hs]

        bqc = np.empty((128, 6), np.float32)
        bqc[:, 0:3] = bq[hs].reshape(3, 128).T
        bqc[:, 3:6] = bk[hs].reshape(3, 128).T

        in_maps.append({
            "xt": np.ascontiguousarray(xt),
            "wqT": np.ascontiguousarray(Wq[hs, :].T.astype(DT_NP)),
            "wkT": np.ascontiguousarray(Wk[hs, :].T.astype(DT_NP)),
            "wvT": wva,
            "bqc": bqc,
            "mt": mt,
        })
    return in_maps


def gather_out(results):
    out = np.empty((BS, SEQ, HID), np.float32)
    for c in range(NCORES):
        b, g = c // 2, c % 2
        ot = results[c]["OT"]  # [6, 64, 2048] f16
        out[b, :, g * DSH:(g + 1) * DSH] = (
            ot.transpose(2, 0, 1).reshape(SEQ, DSH).astype(np.float32)
        )
    return out


def kernel(hidden_states, attention_mask, Wq, bq, Wk, bk, Wv, bv):
    nc = _get_nc()
    in_maps = make_in_maps(hidden_states, attention_mask,
                           Wq, bq, Wk, bk, Wv, bv)
    res = run_bass_kernel_spmd(nc, in_maps, core_ids=list(range(NCORES)))
    return gather_out(res.results)


# revision 20
# speedup vs baseline: 1.0608x; 1.0608x over previous
"""BERT self-attention (BS=4, SEQ=2048, HID=768, NH=12) on 8 NeuronCores.

Sharding: core c -> batch b = c//2, head-group g = c%2 (6 heads each).

Per core, for its batch element and 6 heads (3 pairs j=0..2):
  Q^T/K^T[j] = W_j @ X^T + b  in [d, q] layout (d on partitions), bias
               folded on the DVE during the PSUM->SBUF copy.
  V          = X @ Wv^T + bv  in [k, d] layout, rows scaled by mask m[k],
               projected inline during the first attention loop.
  S^T        = K^T.T-free matmul -> [k_block, q] scores in PSUM
               (two heads row-tiled at PE rows 0-63 / 64-127).
  P^T        = exp(S^T / 8)   (ACT, PSUM->SBUF f16; mask folded into V)
  ctx^T      = V'.T @ P^T accumulated over k blocks (heads col-tiled into
               rows 0-63 / 64-127 of one PSUM tile).
  denom      = mask-column m=1 matmuls, 4-way col-tiled, into a 1-bank
               PSUM tile.
  out        = ctx^T * broadcast(1/denom), f16, DMA'd per q-half.

The emission is software-pipelined: scores/exp for k-block t are emitted
before ctx/denom for t-1, so the ACT engine (the exp throughput bound)
never waits on the PE and vice versa.  QK projections for head-pair j+1
are interleaved into attention loop j.

Host does input transposes (free), sharding, and the final [d,q]->[q,d]
untranspose + concat.
"""

import numpy as np

import concourse.bass as bass
import concourse.tile as tile
from concourse import bacc
from concourse import mybir
from concourse.bass_utils import run_bass_kernel_spmd

F32 = mybir.dt.float32
F16 = mybir.dt.float16
DT_MM = F16          # dtype for matmul operands
DT_NP = np.float16   # matching numpy dtype for host-side input prep

BS, SEQ, HID, NH, HD = 4, 2048, 768, 12, 64
NCORES = 8
HPC = 6          # heads per core
FCH = 6          # 128-row chunks of the 768 contraction dim
DSH = HPC * HD   # 384 output features per core


def _body(tc, xt_d, wq_d, wk_d, wv_d, bqc_d, mt_d, ot_d):
    nc = tc.nc
    Exp = mybir.ActivationFunctionType.Exp

    with tc.tile_pool(name="persist", bufs=1) as P, \
         tc.tile_pool(name="work", bufs=1) as W, \
         tc.tile_pool(name="ps", bufs=1, space="PSUM") as PS:
        # Warm the exp table set ASAP (overlaps the input DMAs).
        dummy = P.tile([1, 1], F32, tag="dummy")
        nc.vector.memset(dummy, 0.0)
        nc.scalar.activation(out=dummy, in_=dummy, func=Exp)

        # ---------------- input DMAs ----------------
        xts = []
        for f in range(FCH):
            t = P.tile([128, SEQ], DT_MM, tag=f"x{f}", name=f"x{f}")
            nc.sync.dma_start(out=t, in_=xt_d[f * 128:(f + 1) * 128, :])
            xts.append(t)

        # Q/K weights arrive as per-head-pair [128,128] slices so the first
        # DMA wave (everything the lead-in projections need) is minimal;
        # j=1,2 slices are emitted last and may arrive during attention j=0.
        wmap = {"q": [[None] * 3 for _ in range(FCH)],
                "k": [[None] * 3 for _ in range(FCH)]}
        for dram, nm in ((wq_d, "q"), (wk_d, "k")):
            for f in range(FCH):
                t = P.tile([128, 128], DT_MM, tag=f"w{nm}{f}0", name=f"w{nm}{f}0")
                nc.sync.dma_start(out=t, in_=dram[f * 128:(f + 1) * 128, 0:128])
                wmap[nm][f][0] = t

        bqc = P.tile([128, 6], F32, tag="bqc")
        nc.sync.dma_start(out=bqc, in_=bqc_d[:, :])

        mtile = P.tile([128, 16], DT_MM, tag="mtile")
        nc.sync.dma_start(out=mtile, in_=mt_d[:, :])
        mtf = P.tile([128, 16], F32, tag="mtf")
        nc.vector.tensor_copy(out=mtf, in_=mtile)

        xt1 = P.tile([1, SEQ], DT_MM, tag="x6")
        nc.sync.dma_start(out=xt1, in_=xt_d[768:769, :])

        wv = []
        for f in range(FCH):
            t = P.tile([128, DSH], DT_MM, tag=f"wv{f}", name=f"wv{f}")
            nc.sync.dma_start(out=t, in_=wv_d[f * 128:(f + 1) * 128, :])
            wv.append(t)
        wvb = P.tile([1, DSH], DT_MM, tag="wvb")
        nc.sync.dma_start(out=wvb, in_=wv_d[768:769, :])

        for dram, nm in ((wq_d, "q"), (wk_d, "k")):
            for j in (1, 2):
                for f in range(FCH):
                    t = P.tile([128, 128], DT_MM, tag=f"w{nm}{f}{j}",
                               name=f"w{nm}{f}{j}")
                    nc.sync.dma_start(
                        out=t, in_=dram[f * 128:(f + 1) * 128,
                                        j * 128:(j + 1) * 128])
                    wmap[nm][f][j] = t

        # ---------------- persistent compute tiles ----------------
        qt = [P.tile([128, SEQ], DT_MM, tag=f"qt{j}", name=f"qt{j}") for j in range(3)]
        kt = [P.tile([128, SEQ], DT_MM, tag=f"kt{j}", name=f"kt{j}") for j in range(3)]
        vt = P.tile([128, 16, DSH], DT_MM, tag="vt")

        def proj_part(nm, j, qc, tag, frange, ps=None, bufs=1):
            """Part of a [128,512] chunk of Q^T[j] or K^T[j]: accumulating
            matmuls over frange; on the last one, a DVE copy folds the
            per-partition bias.  Returns the PSUM tile for continuation."""
            qs = slice(qc * 512, (qc + 1) * 512)
            if ps is None:
                ps = PS.tile([128, 512], F32, tag=tag, name=f"pj{nm}{j}{qc}",
                             bufs=bufs)
            for f in frange:
                nc.tensor.matmul(ps, lhsT=wmap[nm][f][j], rhs=xts[f][:, qs],
                                 start=(f == 0), stop=(f == FCH - 1))
            if frange[-1] == FCH - 1:
                dst = qt[j] if nm == "q" else kt[j]
                bcol = j if nm == "q" else 3 + j
                nc.vector.tensor_scalar_add(out=dst[:, qs], in0=ps,
                                            scalar1=bqc[:, bcol:bcol + 1])
            return ps

        def proj_chunk(nm, j, qc, tag, bufs=1):
            proj_part(nm, j, qc, tag, range(FCH), bufs=bufs)

        def v_chunk(kb):
            """V rows for k-block kb, all 6 heads, mask-scaled into vt."""
            ks = slice(kb * 128, (kb + 1) * 128)
            vps = PS.tile([128, DSH], F32, tag="x", name=f"vps{kb}")
            for f in range(FCH):
                nc.tensor.matmul(vps, lhsT=xts[f][:, ks], rhs=wv[f],
                                 start=(f == 0), stop=False)
            nc.tensor.matmul(vps, lhsT=xt1[:, ks], rhs=wvb,
                             start=False, stop=True)
            nc.vector.tensor_scalar_mul(out=vt[:, kb, :], in0=vps,
                                        scalar1=mtf[:, kb:kb + 1])

        # PE warmup: dense dummy matmuls during the input DMA wait so the
        # HAM clock gate opens (1.2 -> 2.4 GHz) before real work starts.
        warm = P.tile([128, 512], DT_MM, tag="warm")
        nc.vector.memset(warm, 0.0)

        def filler(n, tags=("x", "s")):
            """Always-ready dummy matmuls: bridge PE idle windows so the
            HAM clock gate never re-throttles to 1.2 GHz."""
            for t in range(n):
                wps = PS.tile([128, 512], F32, tag=tags[t % len(tags)],
                              name="wps", bufs=2 if tags[t % len(tags)] == "s" else 1)
                nc.tensor.matmul(wps, lhsT=warm[:, 0:128], rhs=warm,
                                 start=True, stop=True)

        filler(30)

        # Lead-in: project Q^T[0], K^T[0] using the (idle) scores buffers
        # so the chunks double-buffer.
        for nm in ("q", "k"):
            for qc in range(4):
                proj_chunk(nm, 0, qc, "s", bufs=2)

        # ---------------- attention ----------------
        # PSUM tags: s 2x[128,1024] (4 banks) + c [128,1024] (2 banks) +
        # d [97,512] (1 bank) + x [128,512] (1 bank) = 8 banks.
        pending = []   # deferred bc/mul/dma work from the previous drain
        for j in range(3):
            heads = (2 * j, 2 * j + 1)
            # projection chunks for the next head pair, spread over slots
            chunks = []
            if j < 2:
                chunks = [(nm, j + 1, qc) for nm in ("q", "k") for qc in range(4)]
            for qh in range(2):
                q0 = qh * 1024

                cab = PS.tile([128, 1024], F32, tag="c", name="cab")
                dnt = PS.tile([97, 512], F32, tag="d", name="dnt")

                def ctx_dn(ppab, pkb):
                    st, sp_ = (pkb == 0), (pkb == 15)
                    for qq in range(2):
                        osl = slice(qq * 512, (qq + 1) * 512)
                        for i in range(2):
                            nc.tensor.matmul(
                                cab[64 * i:64 * (i + 1), osl],
                                lhsT=vt[:, pkb, heads[i] * 64:(heads[i] + 1) * 64],
                                rhs=ppab[i][:, osl], start=st, stop=sp_,
                                skip_group_check=True)
                    for idx, (i, qq) in enumerate(((0, 0), (1, 0), (0, 1), (1, 1))):
                        osl = slice(qq * 512, (qq + 1) * 512)
                        r = 32 * idx
                        nc.tensor.matmul(dnt[r:r + 1, :],
                                         lhsT=mtile[:, pkb:pkb + 1],
                                         rhs=ppab[i][:, osl],
                                         start=st, stop=sp_,
                                         tile_position=(0, r),
                                         skip_group_check=True)

                LAG = 3  # ctx/dn trail scores/exp by 3 slots so they are
                # never not-ready when they reach the PE (the 4-deep wait
                # queue would otherwise block the engine at loop boundaries)
                lagq = []
                half_state = None
                for kb in range(16):
                    ks = slice(kb * 128, (kb + 1) * 128)
                    # scores for kb (row-tiled head pairs, ping-pong bufs)
                    sab = [PS.tile([128, 1024], F32, tag="s", name="sab", bufs=2)
                           for _ in range(2)]
                    for qq in range(2):
                        qs = slice(q0 + qq * 512, q0 + (qq + 1) * 512)
                        osl = slice(qq * 512, (qq + 1) * 512)
                        for i in range(2):
                            rows = slice(64 * i, 64 * (i + 1))
                            nc.tensor.matmul(sab[i][:, osl],
                                             lhsT=kt[j][rows, ks],
                                             rhs=qt[j][rows, qs],
                                             start=True, stop=True)
                    # exp for kb
                    pab = [W.tile([128, 1024], DT_MM, tag="p", name="ptile", bufs=8)
                           for _ in range(2)]
                    for i in range(2):
                        nc.scalar.activation(out=pab[i], in_=sab[i], func=Exp,
                                             scale=0.125)
                    # V projection for kb (consumed by ctx at slot kb+LAG)
                    if j == 0 and qh == 0:
                        v_chunk(kb)
                    # lagged ctx + denominators
                    lagq.append((pab, kb))
                    if len(lagq) > LAG:
                        ctx_dn(*lagq.pop(0))
                    # deferred normalize/store work from the previous drain
                    if pending and 2 <= kb <= 5:
                        pending.pop(0)()
                    # interleaved projection work for j+1: 3 matmuls per
                    # slot (half a chunk) keeps every slot under the ACT
                    # exp pace
                    if half_state is not None:
                        nm, pj, qc, hps = half_state
                        proj_part(nm, pj, qc, "x", range(3, FCH), ps=hps)
                        half_state = None
                    elif chunks and kb % 2 == 1 and not (j == 0 and qh == 0):
                        nm, pj, qc = chunks.pop(0)
                        hps = proj_part(nm, pj, qc, "x", range(0, 3))
                        half_state = (nm, pj, qc, hps)
                # flush the lagged ctx/dn (also bridges the loop boundary
                # so the PE never idles long enough to re-throttle)
                for ent in lagq:
                    ctx_dn(*ent)
                if half_state is not None:
                    nm, pj, qc, hps = half_state
                    proj_part(nm, pj, qc, "x", range(3, FCH), ps=hps)
                    half_state = None

                # drain (DVE only): free cab/dnt quickly, compute 1/denom.
                # The bc-broadcast matmuls + final multiplies + store DMAs
                # are deferred into slots 2-5 of the next loop so no
                # long-waiting matmul ever clogs the PE wait queue at the
                # loop boundary.
                cts, r16s = [], []
                for i in range(2):
                    ct = W.tile([64, 1024], F32, tag="ct", name="ct", bufs=4)
                    nc.vector.tensor_copy(out=ct, in_=cab[64 * i:64 * (i + 1), :])
                    cts.append(ct)
                    rdf = W.tile([1, 1024], F32, tag="rd", name="rdf", bufs=2)
                    nc.vector.tensor_copy(out=rdf[:, 0:512],
                                          in_=dnt[32 * i:32 * i + 1, :])
                    nc.vector.tensor_copy(out=rdf[:, 512:1024],
                                          in_=dnt[64 + 32 * i:64 + 32 * i + 1, :])
                    rcp = W.tile([1, 1024], F32, tag="rcp", name="rcp", bufs=2)
                    nc.vector.reciprocal_approx_fast(out=rcp, in_=rdf)
                    rcp16 = W.tile([1, 1024], DT_MM, tag="rcp16", name="rcp16", bufs=2)
                    nc.vector.tensor_copy(out=rcp16, in_=rcp)
                    r16s.append(rcp16)
                osts = [W.tile([64, 1024], DT_MM, tag="os", name="ost", bufs=4)
                        for _ in range(2)]

                def normalize_store(i, half, q0=q0, heads=heads,
                                    cts=cts, r16s=r16s, osts=osts):
                    osl = slice(half * 512, (half + 1) * 512)
                    bc = PS.tile([64, 512], F32, tag="x", name="bc")
                    nc.tensor.matmul(bc, lhsT=xt1[:, 0:64],
                                     rhs=r16s[i][:, osl],
                                     start=True, stop=True)
                    nc.vector.tensor_mul(out=osts[i][:, osl],
                                         in0=cts[i][:, osl], in1=bc)
                    if half == 1:
                        nc.sync.dma_start(out=ot_d[heads[i]][:, q0:q0 + 1024],
                                          in_=osts[i])

                pending = [lambda i=i, half=half: normalize_store(i, half)
                           for i in range(2) for half in range(2)]
        # tail: final drain work has no following loop
        for fn in pending:
            fn()


def build_nc():
    nc = bacc.Bacc("TRN2")
    xt_d = nc.declare_dram_parameter("xt", [HID + 1, SEQ], DT_MM, isOutput=False)
    wq_d = nc.declare_dram_parameter("wqT", [HID, DSH], DT_MM, isOutput=False)
    wk_d = nc.declare_dram_parameter("wkT", [HID, DSH], DT_MM, isOutput=False)
    wv_d = nc.declare_dram_parameter("wvT", [HID + 1, DSH], DT_MM, isOutput=False)
    bqc_d = nc.declare_dram_parameter("bqc", [128, 6], F32, isOutput=False)
    mt_d = nc.declare_dram_parameter("mt", [128, 16], DT_MM, isOutput=False)
    ot_d = nc.declare_dram_parameter("OT", [HPC, HD, SEQ], DT_MM, isOutput=True)
    with tile.TileContext(nc) as tc:
        _body(tc, xt_d, wq_d, wk_d, wv_d, bqc_d, mt_d, ot_d)
    nc.finalize()
    return nc


_NC_CACHE = None


def _get_nc():
    global _NC_CACHE
    if _NC_CACHE is None:
        _NC_CACHE = build_nc()
    return _NC_CACHE


def make_in_maps(hidden_states, attention_mask, Wq, bq, Wk, bk, Wv, bv):
    in_maps = []
    for c in range(NCORES):
        b, g = c // 2, c % 2
        hs = slice(g * DSH, (g + 1) * DSH)
        xt = np.empty((HID + 1, SEQ), DT_NP)
        xt[:HID] = hidden_states[b].T
        xt[HID] = 1.0
        m = (attention_mask[b, 0, 0] > -1).astype(DT_NP)
        mt = np.ascontiguousarray(m.reshape(16, 128).T)

        wva = np.empty((HID + 1, DSH), DT_NP)
        wva[:HID] = Wv[hs, :].T
        wva[HID] = bv[hs]

        bqc = np.empty((128, 6), np.float32)
        bqc[:, 0:3] = bq[hs].reshape(3, 128).T
        bqc[:, 3:6] = bk[hs].reshape(3, 128).T

        in_maps.append({
            "xt": np.ascontiguousarray(xt),
            "wqT": np.ascontiguousarray(Wq[hs, :].T.astype(DT_NP)),
            "wkT": np.ascontiguousarray(Wk[hs, :].T.astype(DT_NP)),
            "wvT": wva,
            "bqc": bqc,
            "mt": mt,
        })
    return in_maps


def gather_out(results):
    out = np.empty((BS, SEQ, HID), np.float32)
    for c in range(NCORES):
        b, g = c // 2, c % 2
        ot = results[c]["OT"]  # [6, 64, 2048] f16
        out[b, :, g * DSH:(g + 1) * DSH] = (
            ot.transpose(2, 0, 1).reshape(SEQ, DSH).astype(np.float32)
        )
    return out


def kernel(hidden_states, attention_mask, Wq, bq, Wk, bk, Wv, bv):
    nc = _get_nc()
    in_maps = make_in_maps(hidden_states, attention_mask,
                           Wq, bq, Wk, bk, Wv, bv)
    res = run_bass_kernel_spmd(nc, in_maps, core_ids=list(range(NCORES)))
    return gather_out(res.results)
